# revision 1
# baseline (speedup 1.0000x reference)
"""Trainium2 Bass kernel for nn_Attention_55499567399068.

Episode-attention block: per (batch, nway) pair of [64, 512] blocks:
  q/k/v linear projections -> scaled dot-product attention over nshot ->
  reduce_att MLP producing per-row weights -> weighted sum of context rows.

Sharding: pure data parallel over batch across 8 NeuronCores (32 episodes each).
Per core: 256 independent (b, n) pairs, processed in 32 superblocks of 8 pairs.

Math restructuring (exact, up to fp precision):
  - 1/sqrt(d) folded into Wq, bq on host.
  - Softmax without max-subtraction (scores are O(1)): E = exp(S) on ScalarE
    with the row-sum Z as the same instruction's accum_out; A = E * (1/Z) is a
    cheap per-partition scale in E-natural layout.
       hid = Wr1 @ C^T; leaky+br1-bias fused into one ScalarE op
       w = hid^T @ Wr2 + br2;  g = A^T @ w;  out^T[h] = sum_k v[k,h] * g[k]
  - bq, bk added via per-partition activation bias on the PSUM->SBUF copy
    (q^T/k^T layouts have h on partitions); bv added via a broadcast-tile
    tensor_tensor add on the v copy (v natural layout has h on free).

Layout strategy: inputs are loaded naturally ([row, d]), cast to bf16, and
transposed to [d, row] with the DMA xbar (2-byte transpose engine) so the
d-contraction projections run at full PE rate.
"""

import sys

sys.path.insert(0, "/opt/trn_rl_repo")

import ml_dtypes
import numpy as np

import concourse.bass as bass
import concourse.tile as tile
from concourse import bacc, mybir
from concourse.bass_utils import run_bass_kernel_spmd

F32 = mybir.dt.float32
BF16 = mybir.dt.bfloat16
BF16_NP = ml_dtypes.bfloat16

BS, NWAY, NSHOT, D = 256, 8, 64, 512
NCORES = 8
BS_SH = BS // NCORES  # 32 episodes per core
NPAIR = BS_SH * NWAY  # 256 pairs per core
SUPER = 8  # pairs per superblock
NSB = NPAIR // SUPER  # 32 superblocks
ROWS_SB = SUPER * NSHOT  # 512 rows per superblock
LEAK = 0.01
AT = mybir.ActivationFunctionType
ALU = mybir.AluOpType

BR2_VAL = [0.0]  # captured at build time as an immediate
SUB = {"act3d": True, "br1grp": True, "widelrelu": True, "zrowmm": True, "rank1": True}


def build_nc(repeat=1, cast_dma=True, n_sb=NSB, lrelu=True, widehid=False, xbar3d=False):
    nc = bacc.Bacc("TRN2", target_bir_lowering=False)

    xq = nc.dram_tensor("xq", [NPAIR * NSHOT, D], F32, kind="ExternalInput")
    xk = nc.dram_tensor("xk", [NPAIR * NSHOT, D], F32, kind="ExternalInput")
    xv = nc.dram_tensor("xv", [NPAIR * NSHOT, D], F32, kind="ExternalInput")
    wqT_d = nc.dram_tensor("wqT", [D, D], BF16, kind="ExternalInput")  # [d, h]
    wkT_d = nc.dram_tensor("wkT", [D, D], BF16, kind="ExternalInput")
    wvT_d = nc.dram_tensor("wvT", [D, D], BF16, kind="ExternalInput")
    wr1T_d = nc.dram_tensor("wr1T", [D, 64], BF16, kind="ExternalInput")  # [h, m]
    wr2T_d = nc.dram_tensor("wr2T", [64, 1], BF16, kind="ExternalInput")  # [m, 1]
    br1b_d = nc.dram_tensor("br1b", [128, 64], BF16, kind="ExternalInput")
    bvb_d = nc.dram_tensor("bvb", [128, D], BF16, kind="ExternalInput")
    bq_d = nc.dram_tensor("bq", [128, 4], F32, kind="ExternalInput")
    bk_d = nc.dram_tensor("bk", [128, 4], F32, kind="ExternalInput")
    br1c_d = nc.dram_tensor("br1c", [64, 1], F32, kind="ExternalInput")
    out_d = nc.dram_tensor("out", [NPAIR, D], F32, kind="ExternalOutput")

    with tile.TileContext(nc) as tc:
        import contextlib

        ctx = contextlib.ExitStack()
        with ctx:
            const_pool = ctx.enter_context(tc.tile_pool(name="const", bufs=1))
            ld_pool = ctx.enter_context(tc.tile_pool(name="loads", bufs=3))
            xt_pool = ctx.enter_context(tc.tile_pool(name="xt", bufs=2))
            proj_pool = ctx.enter_context(tc.tile_pool(name="projs", bufs=2))
            mid_pool = ctx.enter_context(tc.tile_pool(name="mid", bufs=2))
            out_pool = ctx.enter_context(tc.tile_pool(name="outs", bufs=2))
            psA = ctx.enter_context(tc.tile_pool(name="psA", bufs=2, space="PSUM"))
            psS = ctx.enter_context(tc.tile_pool(name="psS", bufs=3, space="PSUM"))
            psC = ctx.enter_context(tc.tile_pool(name="psC", bufs=2, space="PSUM"))
            psB = ctx.enter_context(tc.tile_pool(name="psB", bufs=1, space="PSUM"))

            wqT = const_pool.tile([128, 4 * D], BF16, tag="wqT")
            wkT = const_pool.tile([128, 4 * D], BF16, tag="wkT")
            wvT = const_pool.tile([128, 4 * D], BF16, tag="wvT")
            wr1T = const_pool.tile([128, 4 * 64], BF16, tag="wr1T")
            wr2T = const_pool.tile([64, 1], BF16, tag="wr2T")
            br1b = const_pool.tile([128, 64], BF16, tag="br1b")
            bvb = const_pool.tile([128, D], BF16, tag="bvb")
            bqs = const_pool.tile([128, 4], F32, tag="bqs")
            bks = const_pool.tile([128, 4], F32, tag="bks")
            br1c = const_pool.tile([64, 1], F32, tag="br1c")

            def load_consts():
                nc.sync.dma_start(
                    wqT[:].rearrange("p (dc h) -> p dc h", dc=4),
                    wqT_d[:, :].rearrange("(dc p) h -> p dc h", p=128),
                )
                nc.sync.dma_start(
                    wkT[:].rearrange("p (dc h) -> p dc h", dc=4),
                    wkT_d[:, :].rearrange("(dc p) h -> p dc h", p=128),
                )
                nc.sync.dma_start(
                    wvT[:].rearrange("p (dc h) -> p dc h", dc=4),
                    wvT_d[:, :].rearrange("(dc p) h -> p dc h", p=128),
                )
                nc.sync.dma_start(
                    wr1T[:].rearrange("p (hc m) -> p hc m", hc=4),
                    wr1T_d[:, :].rearrange("(hc p) m -> p hc m", p=128),
                )
                nc.sync.dma_start(wr2T[:], wr2T_d[:, :])
                nc.sync.dma_start(br1b[:], br1b_d[:, :])
                nc.sync.dma_start(bvb[:], bvb_d[:, :])
                nc.sync.dma_start(bqs[:], bq_d[:, :])
                nc.sync.dma_start(bks[:], bk_d[:, :])
                nc.sync.dma_start(br1c[:], br1c_d[:, :])

            def emit_superblock(sb):
                # ---------- Phase A: load + cast + transpose ----------
                xts = {}
                for name, src in (("q", xq), ("k", xk), ("v", xv)):
                    src_ap = src[bass.ts(sb, ROWS_SB), :].rearrange(
                        "(r p) d -> p r d", p=128
                    )
                    if cast_dma:
                        xbf = ld_pool.tile([128, 4 * D], BF16, tag=f"xbf{name}")
                        nc.gpsimd.dma_start(
                            xbf[:].rearrange("p (r d) -> p r d", r=4), src_ap
                        )
                    else:
                        xf32 = ld_pool.tile([128, 4 * D], F32, tag=f"xf{name}")
                        nc.sync.dma_start(
                            xf32[:].rearrange("p (r d) -> p r d", r=4), src_ap
                        )
                        xbf = ld_pool.tile([128, 4 * D], BF16, tag=f"xbf{name}")
                        nc.vector.tensor_copy(xbf[:], xf32[:])
                    xt = xt_pool.tile([128, 4 * D], BF16, tag=f"xt{name}")
                    if xbar3d:
                        xt4 = xt[:].rearrange("p (dc rb i) -> p dc rb i", dc=4, rb=4)
                        for r in range(4):
                            # one xbar DMA transposes a [128, 512] row-block:
                            # out[p, dc, i] = in[i, dc*128+p]
                            nc.sync.dma_start(
                                xt4[:, :, r, :],
                                xbf[:, r * 512 : (r + 1) * 512],
                                transpose=True,
                            )
                    else:
                        for r in range(4):
                            for dc in range(4):
                                nc.sync.dma_start(
                                    xt[
                                        :,
                                        dc * 512 + r * 128 : dc * 512 + (r + 1) * 128,
                                    ],
                                    xbf[
                                        :,
                                        r * 512 + dc * 128 : r * 512 + (dc + 1) * 128,
                                    ],
                                    transpose=True,
                                )
                    xts[name] = xt

                # ---------- Phase B: projections ----------
                qTs = proj_pool.tile([128, 4 * 512], BF16, tag="qTs")
                kTs = proj_pool.tile([128, 4 * 512], BF16, tag="kTs")
                vS = proj_pool.tile([128, 4 * 512], BF16, tag="vS")
                for dst, w_t, x_t, bias_t in (
                    (qTs, wqT, xts["q"], bqs),
                    (kTs, wkT, xts["k"], bks),
                ):
                    for hc in range(4):
                        ps = psA.tile([128, 512], F32, tag="proj")
                        for dc in range(4):
                            nc.tensor.matmul(
                                ps[:],
                                lhsT=w_t[
                                    :, dc * 512 + hc * 128 : dc * 512 + (hc + 1) * 128
                                ],
                                rhs=x_t[:, dc * 512 : (dc + 1) * 512],
                                start=(dc == 0),
                                stop=(dc == 3),
                            )
                        nc.scalar.activation(
                            dst[:, hc * 512 : (hc + 1) * 512],
                            ps[:],
                            AT.Identity,
                            bias=bias_t[:, hc : hc + 1],
                        )
                for r in range(4):
                    ps = psA.tile([128, 512], F32, tag="proj")
                    for dc in range(4):
                        nc.tensor.matmul(
                            ps[:],
                            lhsT=xts["v"][
                                :, dc * 512 + r * 128 : dc * 512 + (r + 1) * 128
                            ],
                            rhs=wvT[:, dc * 512 : (dc + 1) * 512],
                            start=(dc == 0),
                            stop=(dc == 3),
                        )
                    nc.vector.tensor_tensor(
                        vS[:, r * 512 : (r + 1) * 512], ps[:], bvb[:], op=ALU.add
                    )

                # ---------- Phase C ----------
                bat = psB.tile([128, 512], F32, tag="batch")
                Zb = mid_pool.tile([64, 8], F32, tag="Zb")

                # C1: scores, 8 pairs as regions of one bank
                s_all = psS.tile([64, 512], F32, tag="sh")
                for pr in range(SUPER):
                    for hc in range(4):
                        nc.tensor.matmul(
                            s_all[:, pr * 64 : (pr + 1) * 64],
                            lhsT=qTs[:, hc * 512 + pr * 64 : hc * 512 + (pr + 1) * 64],
                            rhs=kTs[:, hc * 512 + pr * 64 : hc * 512 + (pr + 1) * 64],
                            start=(hc == 0),
                            stop=(hc == 3),
                        )
                # C2: E = exp(S), Z = rowsum(E)
                e2s = []
                for pp in range(4):
                    e2 = mid_pool.tile([64, 128], BF16, tag=f"e2_{pp}")
                    e2s.append(e2)
                for pr in range(SUPER):
                    pp, half = pr // 2, pr % 2
                    nc.scalar.activation(
                        e2s[pp][:, half * 64 : (half + 1) * 64],
                        s_all[:, pr * 64 : (pr + 1) * 64],
                        AT.Exp,
                        accum_out=Zb[:, pr : pr + 1],
                    )
                # C2b: normalize in place: A = E * (1/Z), per-partition scale
                rT = mid_pool.tile([64, 8], F32, tag="rT")
                nc.vector.reciprocal(rT[:], Zb[:])
                for pr in range(SUPER):
                    pp, half = pr // 2, pr % 2
                    nc.vector.tensor_scalar(
                        e2s[pp][:, half * 64 : (half + 1) * 64],
                        e2s[pp][:, half * 64 : (half + 1) * 64],
                        rT[:, pr : pr + 1],
                        None,
                        op0=ALU.mult,
                    )
                # C3: A^T via xbar
                ets = []
                for pp in range(4):
                    et = mid_pool.tile([128, 64], BF16, tag=f"et_{pp}")
                    nc.sync.dma_start(et[:], e2s[pp][:], transpose=True)
                    ets.append(et)

                ys_all = mid_pool.tile([64, 512], BF16, tag="ys_all")
                lr = AT.Lrelu if lrelu else AT.Relu
                if widehid:
                    # C4: C~^T per pair into combined [p, hc, (pr q)] tile
                    cts_all = mid_pool.tile([128, 4 * 512], BF16, tag="cts_all")
                    cts3 = cts_all[:].rearrange("p (hc prq) -> p hc prq", hc=4)
                    for pr in range(SUPER):
                        pp, half = pr // 2, pr % 2
                        ctp = psC.tile([128, 256], F32, tag="ct")
                        for hc in range(4):
                            nc.tensor.matmul(
                                ctp[:, hc * 64 : (hc + 1) * 64],
                                lhsT=vS[
                                    half * 64 : (half + 1) * 64,
                                    pp * 512 + hc * 128 : pp * 512 + (hc + 1) * 128,
                                ],
                                rhs=ets[pp][half * 64 : (half + 1) * 64, :],
                                start=True,
                                stop=True,
                            )
                        dst = cts3[:, :, pr * 64 : (pr + 1) * 64]
                        src3 = ctp[:].rearrange("p (hc q) -> p hc q", hc=4)
                        if pr % 2 == 0 or not SUB["act3d"]:
                            nc.vector.tensor_copy(dst, src3)
                        else:
                            nc.scalar.activation(dst, src3, AT.Copy)
                    # C5: hid = 4 wide matmuls (A is normalized, C is true context)
                    hid_all = psS.tile([64, 512], F32, tag="sh")
                    for hc in range(4):
                        nc.tensor.matmul(
                            hid_all[:],
                            lhsT=wr1T[:, hc * 64 : (hc + 1) * 64],
                            rhs=cts_all[:, hc * 512 : (hc + 1) * 512],
                            start=(hc == 0),
                            stop=(hc == 3),
                        )
                    # C6: leaky relu (+ br1 per-partition bias) over all pairs
                    nc.scalar.activation(
                        ys_all[:], hid_all[:], lr, bias=br1c[:], alpha=LEAK
                    )
                else:
                    # v1-style per-pair C~/hid/leaky
                    for pr in range(SUPER):
                        pp, half = pr // 2, pr % 2
                        ctp = psC.tile([128, 256], F32, tag="ct")
                        for hc in range(4):
                            nc.tensor.matmul(
                                ctp[:, hc * 64 : (hc + 1) * 64],
                                lhsT=vS[
                                    half * 64 : (half + 1) * 64,
                                    pp * 512 + hc * 128 : pp * 512 + (hc + 1) * 128,
                                ],
                                rhs=ets[pp][half * 64 : (half + 1) * 64, :],
                                start=True,
                                stop=True,
                            )
                        cts = mid_pool.tile([128, 256], BF16, tag="cts")
                        nc.vector.tensor_copy(cts[:], ctp[:])
                        hid_t = psS.tile([64, 512], F32, tag="sh")
                        hid = hid_t[:, 0:64]
                        for hc in range(4):
                            nc.tensor.matmul(
                                hid,
                                lhsT=wr1T[:, hc * 64 : (hc + 1) * 64],
                                rhs=cts[:, hc * 64 : (hc + 1) * 64],
                                start=(hc == 0),
                                stop=(hc == 3),
                            )
                        nc.scalar.activation(
                            ys_all[:, pr * 64 : (pr + 1) * 64], hid, lr,
                            bias=br1c[:], alpha=LEAK,
                        )

                # C7: w~^T columns
                for pr in range(SUPER):
                    nc.tensor.matmul(
                        bat[0:64, pr : pr + 1],
                        lhsT=ys_all[:, pr * 64 : (pr + 1) * 64],
                        rhs=wr2T[:],
                        start=True,
                        stop=True,
                    )
                # C8: w = w~ + br2 (A was normalized, so no Z scaling here)
                wh = mid_pool.tile([64, 8], BF16, tag="wh")
                nc.vector.tensor_scalar(
                    wh[:], bat[0:64, 0:8], float(BR2_VAL[0]), None, op0=ALU.add
                )
                # C9: g = E^T @ w^
                for pr in range(SUPER):
                    pp, half = pr // 2, pr % 2
                    nc.tensor.matmul(
                        bat[0:64, 8 + pr : 9 + pr],
                        lhsT=e2s[pp][:, half * 64 : (half + 1) * 64],
                        rhs=wh[:, pr : pr + 1],
                        start=True,
                        stop=True,
                    )
                gS = mid_pool.tile([128, 8], BF16, tag="gS")
                nc.vector.tensor_copy(gS[0:64, :], bat[0:64, 8:16])
                nc.vector.tensor_copy(gS[64:128, :], bat[0:64, 8:16])
                # C10: out^T chunks
                for pr in range(SUPER):
                    pp, half = pr // 2, pr % 2
                    for hc in range(4):
                        nc.tensor.matmul(
                            bat[:, 16 + pr * 4 + hc : 17 + pr * 4 + hc],
                            lhsT=vS[
                                half * 64 : (half + 1) * 64,
                                pp * 512 + hc * 128 : pp * 512 + (hc + 1) * 128,
                            ],
                            rhs=gS[half * 64 : (half + 1) * 64, pr : pr + 1],
                            start=True,
                            stop=True,
                        )
                outTs = out_pool.tile([128, 32], F32, tag="outTs")
                nc.scalar.activation(outTs[:], bat[:, 16:48], AT.Copy)
                outN = out_pool.tile([32, 128], F32, tag="outN")
                for b in range(4):
                    nc.vector.transpose(
                        outN[0:32, b * 32 : (b + 1) * 32],
                        outTs[b * 32 : (b + 1) * 32, 0:32],
                    )
                nc.sync.dma_start(
                    out_d[bass.ts(sb, SUPER), :].rearrange(
                        "pr (hc c) -> (pr hc) c", hc=4
                    ),
                    outN[:],
                )

            def body(_iv=None):
                load_consts()
                for sb in range(n_sb):
                    emit_superblock(sb)

            if repeat == 1:
                body()
            else:
                with tc.For_i(0, repeat, 1) as _iv:
                    body(_iv)

    nc.compile()
    return nc


def prep_in_maps(query, key, value, Wq, bq, Wk, bk, Wv, bv, Wr1, br1, Wr2, br2):
    """Host-side prep: shard + weight transforms. Returns in_maps list of 8 dicts."""
    s = 1.0 / np.sqrt(np.float32(D))
    wqT = (Wq * s).T.astype(BF16_NP).copy()  # [d, h]
    wkT = Wk.T.astype(BF16_NP).copy()
    wvT = Wv.T.astype(BF16_NP).copy()
    wr1T = Wr1.T.astype(BF16_NP).copy()  # [h, m]
    wr2T = Wr2.T.astype(BF16_NP).copy()  # [m, 1]
    br1b = np.tile(br1[None, :].astype(np.float32), (128, 1)).astype(BF16_NP)
    bvb = np.tile(bv[None, :].astype(np.float32), (128, 1)).astype(BF16_NP)
    bqv = (bq * s).astype(np.float32).reshape(4, 128).T.copy()  # [p, hc]
    bkv = bk.astype(np.float32).reshape(4, 128).T.copy()
    br1c = br1.astype(np.float32).reshape(64, 1).copy()
    BR2_VAL[0] = float(br2[0])

    in_maps = []
    for c in range(NCORES):
        sl = slice(c * BS_SH, (c + 1) * BS_SH)
        in_maps.append(
            {
                "xq": np.ascontiguousarray(query[sl]).reshape(NPAIR * NSHOT, D),
                "xk": np.ascontiguousarray(key[sl]).reshape(NPAIR * NSHOT, D),
                "xv": np.ascontiguousarray(value[sl]).reshape(NPAIR * NSHOT, D),
                "wqT": wqT,
                "wkT": wkT,
                "wvT": wvT,
                "wr1T": wr1T,
                "wr2T": wr2T,
                "br1b": br1b,
                "bvb": bvb,
                "bq": bqv,
                "bk": bkv,
                "br1c": br1c,
            }
        )
    return in_maps


_nc_cache = {}


def kernel(**inputs):
    in_maps = prep_in_maps(**{k: np.asarray(v) for k, v in inputs.items()})
    key = ("k", 1, BR2_VAL[0])
    if key not in _nc_cache:
        _nc_cache[key] = build_nc(repeat=1)
    nc = _nc_cache[key]
    res = run_bass_kernel_spmd(nc, in_maps, core_ids=list(range(NCORES)))
    outs = [res.results[c]["out"].reshape(BS_SH, NWAY, D) for c in range(NCORES)]
    return np.concatenate(outs, axis=0).astype(np.float32)



# revision 3
# speedup vs baseline: 2.6606x; 2.6606x over previous
"""Trainium2 Bass kernel for nn_Attention_55499567399068 (v2).

Episode-attention block, data-parallel over batch across 8 NeuronCores
(32 episodes => 256 independent (b, n) pairs per core), processed in 32
superblocks of 8 pairs (512 rows).

v2 restructuring vs v1 (the prior baseline):
  - The v-projection is eliminated algebraically. With A = softmax(S):
      B^T  = Xv^T A^T                      (natural-layout Xv as lhsT)
      hidT = (Wr1 Wv) B^T + (br1 + Wr1 bv) (bias via activation, per-partition)
      w    = lrelu(hidT)^T wr2 + br2
      g    = A^T w ;  z = Xv^T g ;  Sw = sum(w)
      out  = Wv z + bv * Sw                (bias as a K=1 matmul into PSUM)
  - DMA xbar transposes are batched: one DMA per tensor per superblock
    (q, k inputs and the merged E^T), instead of 16+ small ones. This kills
    the sync-engine DMA-issue storm that dominated v1 (~2.1ms busy).
  - hid is computed with 4 wide N=512 matmuls over all 8 pairs at once.

Softmax without max-subtraction (scores are O(1)): E = exp(S) on ScalarE with
row-sum Z as accum_out; A = E * (1/Z) per-partition scale on DVE.
1/sqrt(d) folded into Wq, bq on host.
"""

import sys

sys.path.insert(0, "/opt/trn_rl_repo")

import ml_dtypes
import numpy as np

import concourse.bass as bass
import concourse.tile as tile
from concourse import bacc, mybir
from concourse.bass_utils import run_bass_kernel_spmd

F32 = mybir.dt.float32
BF16 = mybir.dt.bfloat16
BF16_NP = ml_dtypes.bfloat16

BS, NWAY, NSHOT, D = 256, 8, 64, 512
NCORES = 8
BS_SH = BS // NCORES  # 32 episodes per core
NPAIR = BS_SH * NWAY  # 256 pairs per core
SUPER = 8  # pairs per superblock
NSB = NPAIR // SUPER  # 32 superblocks
ROWS_SB = SUPER * NSHOT  # 512 rows per superblock
LEAK = 0.01
AT = mybir.ActivationFunctionType
ALU = mybir.AluOpType

BR2_VAL = [0.0]  # captured at build time as an immediate


def build_nc(
    repeat=1,
    n_sb=NSB,
    big_xpose=True,
    k1_mm=True,
    big_e=True,
    phases=99,
    z_mm=True,
    vbufs=2,
    midbufs=2,
    xtbufs=2,
    projbufs=2,
    ldbufs=3,
    outbufs=2,
):
    nc = bacc.Bacc("TRN2", target_bir_lowering=False)

    xq = nc.dram_tensor("xq", [NPAIR * NSHOT, D], F32, kind="ExternalInput")
    xk = nc.dram_tensor("xk", [NPAIR * NSHOT, D], F32, kind="ExternalInput")
    xv = nc.dram_tensor("xv", [NPAIR * NSHOT, D], F32, kind="ExternalInput")
    wqT_d = nc.dram_tensor("wqT", [D, D], BF16, kind="ExternalInput")  # [d, h]
    wkT_d = nc.dram_tensor("wkT", [D, D], BF16, kind="ExternalInput")
    wvT_d = nc.dram_tensor("wvT", [D, D], BF16, kind="ExternalInput")
    w1vT_d = nc.dram_tensor("w1vT", [D, 64], BF16, kind="ExternalInput")  # [d, m]
    wr2T_d = nc.dram_tensor("wr2T", [64, 1], BF16, kind="ExternalInput")  # [m, 1]
    bq_d = nc.dram_tensor("bq", [128, 4], F32, kind="ExternalInput")
    bk_d = nc.dram_tensor("bk", [128, 4], F32, kind="ExternalInput")
    b1c_d = nc.dram_tensor("b1c", [64, 1], F32, kind="ExternalInput")
    bvr_d = nc.dram_tensor("bvr", [1, D], BF16, kind="ExternalInput")
    ones_d = nc.dram_tensor("ones", [64, 1], BF16, kind="ExternalInput")
    out_d = nc.dram_tensor("out", [NPAIR, D], F32, kind="ExternalOutput")

    with tile.TileContext(nc) as tc:
        import contextlib

        ctx = contextlib.ExitStack()
        with ctx:
            const_pool = ctx.enter_context(tc.tile_pool(name="const", bufs=1))
            ld_pool = ctx.enter_context(tc.tile_pool(name="loads", bufs=ldbufs))
            v_pool = ctx.enter_context(tc.tile_pool(name="vload", bufs=vbufs))
            xt_pool = ctx.enter_context(tc.tile_pool(name="xt", bufs=xtbufs))
            proj_pool = ctx.enter_context(tc.tile_pool(name="projs", bufs=projbufs))
            mid_pool = ctx.enter_context(tc.tile_pool(name="mid", bufs=midbufs))
            out_pool = ctx.enter_context(tc.tile_pool(name="outs", bufs=outbufs))
            psA = ctx.enter_context(tc.tile_pool(name="psA", bufs=2, space="PSUM"))
            psS = ctx.enter_context(tc.tile_pool(name="psS", bufs=3, space="PSUM"))
            psC = ctx.enter_context(tc.tile_pool(name="psC", bufs=2, space="PSUM"))
            psB = ctx.enter_context(tc.tile_pool(name="psB", bufs=1, space="PSUM"))

            wqT = const_pool.tile([128, 4 * D], BF16, tag="wqT")
            wkT = const_pool.tile([128, 4 * D], BF16, tag="wkT")
            wvT = const_pool.tile([128, 4 * D], BF16, tag="wvT")
            w1vT = const_pool.tile([128, 4 * 64], BF16, tag="w1vT")
            wr2T = const_pool.tile([64, 1], BF16, tag="wr2T")
            bqs = const_pool.tile([128, 4], F32, tag="bqs")
            bks = const_pool.tile([128, 4], F32, tag="bks")
            b1c = const_pool.tile([64, 1], F32, tag="b1c")
            bvr = const_pool.tile([1, D], BF16, tag="bvr")
            ones = const_pool.tile([64, 1], BF16, tag="ones")

            def load_consts():
                nc.sync.dma_start(
                    wqT[:].rearrange("p (dc h) -> p dc h", dc=4),
                    wqT_d[:, :].rearrange("(dc p) h -> p dc h", p=128),
                )
                nc.sync.dma_start(
                    wkT[:].rearrange("p (dc h) -> p dc h", dc=4),
                    wkT_d[:, :].rearrange("(dc p) h -> p dc h", p=128),
                )
                nc.sync.dma_start(
                    wvT[:].rearrange("p (dc h) -> p dc h", dc=4),
                    wvT_d[:, :].rearrange("(dc p) h -> p dc h", p=128),
                )
                nc.sync.dma_start(
                    w1vT[:].rearrange("p (dc m) -> p dc m", dc=4),
                    w1vT_d[:, :].rearrange("(dc p) m -> p dc m", p=128),
                )
                nc.sync.dma_start(wr2T[:], wr2T_d[:, :])
                nc.sync.dma_start(bqs[:], bq_d[:, :])
                nc.sync.dma_start(bks[:], bk_d[:, :])
                nc.sync.dma_start(b1c[:], b1c_d[:, :])
                nc.sync.dma_start(bvr[:], bvr_d[:, :])
                nc.sync.dma_start(ones[:], ones_d[:, :])

            def emit_superblock(sb):
                # ---------- Phase A: load (cast f32->bf16 in DMA) ----------
                xbfs = {}
                for name, src, pool in (
                    ("q", xq, ld_pool),
                    ("k", xk, ld_pool),
                    ("v", xv, v_pool),
                ):
                    src_ap = src[bass.ts(sb, ROWS_SB), :].rearrange(
                        "(r p) d -> p r d", p=128
                    )
                    xbf = pool.tile([128, 4 * D], BF16, tag=f"xbf{name}")
                    nc.gpsimd.dma_start(
                        xbf[:].rearrange("p (r d) -> p r d", r=4), src_ap
                    )
                    xbfs[name] = xbf

                if phases <= 1:
                    return
                # ---------- Phase A2: transpose q, k via DMA xbar ----------
                # xbf layout: [p, (r, d)] with row = r*128+p.
                xts = {}
                for name in ("q", "k"):
                    xt = xt_pool.tile([128, 4 * D], BF16, tag=f"xt{name}")
                    if big_xpose:
                        # One DMA: out[p, c, i] = in[i, c*128+p], c = r*4+dc.
                        # => xt[p, (r, dc, i)] = x^T[dc*128+p, r*128+i]
                        nc.sync.dma_start(
                            xt[:].rearrange("p (c i) -> p c i", c=16),
                            xbfs[name][:],
                            transpose=True,
                        )
                    else:
                        # 4 DMAs, dc-major layout: xt[p, (dc, r, i)]
                        xt4 = xt[:].rearrange("p (dc r i) -> p dc r i", dc=4, r=4)
                        for r in range(4):
                            nc.sync.dma_start(
                                xt4[:, :, r, :],
                                xbfs[name][:, r * 512 : (r + 1) * 512],
                                transpose=True,
                            )
                    xts[name] = xt

                if phases <= 2:
                    return
                # ---------- Phase B: q/k projections ----------
                # qTs/kTs layout: [h-in-chunk p, (hc, row)] with row = r*128+i.
                qTs = proj_pool.tile([128, 4 * 512], BF16, tag="qTs")
                kTs = proj_pool.tile([128, 4 * 512], BF16, tag="kTs")
                for dst, w_t, x_t, bias_t in (
                    (qTs, wqT, xts["q"], bqs),
                    (kTs, wkT, xts["k"], bks),
                ):
                    if big_xpose:
                        x3 = x_t[:].rearrange("p (r dc i) -> p dc r i", r=4, dc=4)
                    else:
                        x3 = x_t[:].rearrange("p (dc ri) -> p dc ri", dc=4)
                    for hc in range(4):
                        ps = psA.tile([128, 512], F32, tag="proj")
                        for dc in range(4):
                            nc.tensor.matmul(
                                ps[:],
                                lhsT=w_t[
                                    :, dc * 512 + hc * 128 : dc * 512 + (hc + 1) * 128
                                ],
                                rhs=x3[:, dc],
                                start=(dc == 0),
                                stop=(dc == 3),
                            )
                        nc.scalar.activation(
                            dst[:, hc * 512 : (hc + 1) * 512],
                            ps[:],
                            AT.Identity,
                            bias=bias_t[:, hc : hc + 1],
                        )

                if phases <= 3:
                    return
                # ---------- Phase C ----------
                bat = psB.tile([128, 512], F32, tag="batch")

                # C1: scores S = q k^T (pair pr occupies cols pr*64..)
                s_all = psS.tile([64, 512], F32, tag="sh")
                for pr in range(SUPER):
                    for hc in range(4):
                        nc.tensor.matmul(
                            s_all[:, pr * 64 : (pr + 1) * 64],
                            lhsT=qTs[:, hc * 512 + pr * 64 : hc * 512 + (pr + 1) * 64],
                            rhs=kTs[:, hc * 512 + pr * 64 : hc * 512 + (pr + 1) * 64],
                            start=(hc == 0),
                            stop=(hc == 3),
                        )

                if phases <= 4:
                    return
                # C2: E = exp(S), Z = rowsum(E); then A = E * (1/Z) in place
                e_all = mid_pool.tile([64, 512], BF16, tag="e_all")
                Zb = mid_pool.tile([64, 8], F32, tag="Zb")
                for pr in range(SUPER):
                    nc.scalar.activation(
                        e_all[:, pr * 64 : (pr + 1) * 64],
                        s_all[:, pr * 64 : (pr + 1) * 64],
                        AT.Exp,
                        accum_out=Zb[:, pr : pr + 1],
                    )
                rT = mid_pool.tile([64, 8], F32, tag="rT")
                nc.vector.reciprocal(rT[:], Zb[:])
                for pr in range(SUPER):
                    nc.vector.tensor_scalar(
                        e_all[:, pr * 64 : (pr + 1) * 64],
                        e_all[:, pr * 64 : (pr + 1) * 64],
                        rT[:, pr : pr + 1],
                        None,
                        op0=ALU.mult,
                    )

                if phases <= 5:
                    return
                # C3: A^T via one xbar DMA: etall[(half,k), pp, q] = A_pr[q, k]
                etall = mid_pool.tile([128, 4 * 64], BF16, tag="etall")
                if big_e:
                    nc.sync.dma_start(
                        etall[:].rearrange("p (c i) -> p c i", c=4),
                        e_all[:],
                        transpose=True,
                    )
                else:
                    for pp in range(4):
                        nc.sync.dma_start(
                            etall[:, pp * 64 : (pp + 1) * 64],
                            e_all[:, pp * 128 : (pp + 1) * 128],
                            transpose=True,
                        )
                et3 = etall[:].rearrange("p (pp q) -> p pp q", pp=4)

                if phases <= 6:
                    return
                # C4: B^T = Xv^T A^T per pair (natural-layout Xv as lhsT)
                # bTall layout: [d-in-chunk p, (dc, pr, q)]
                bTall = mid_pool.tile([128, 4 * 512], BF16, tag="bTall")
                bT3 = bTall[:].rearrange("p (dc prq) -> p dc prq", dc=4)
                xv_t = xbfs["v"]
                for pr in range(SUPER):
                    pp, half = pr // 2, pr % 2
                    hs = slice(half * 64, (half + 1) * 64)
                    ctp = psC.tile([128, 256], F32, tag="ct")
                    for dc in range(4):
                        nc.tensor.matmul(
                            ctp[:, dc * 64 : (dc + 1) * 64],
                            lhsT=xv_t[
                                hs, pp * 512 + dc * 128 : pp * 512 + (dc + 1) * 128
                            ],
                            rhs=et3[hs, pp, :],
                            start=True,
                            stop=True,
                        )
                    dst = bT3[:, :, pr * 64 : (pr + 1) * 64]
                    src3 = ctp[:].rearrange("p (dc q) -> p dc q", dc=4)
                    if pr % 2 == 0:
                        nc.vector.tensor_copy(dst, src3)
                    else:
                        nc.scalar.activation(dst, src3, AT.Copy)

                if phases <= 7:
                    return
                # C5: hidT = W1v B^T (4 wide matmuls over all pairs)
                hid_all = psS.tile([64, 512], F32, tag="sh")
                for dc in range(4):
                    nc.tensor.matmul(
                        hid_all[:],
                        lhsT=w1vT[:, dc * 64 : (dc + 1) * 64],
                        rhs=bTall[:, dc * 512 : (dc + 1) * 512],
                        start=(dc == 0),
                        stop=(dc == 3),
                    )
                # C6: ys = lrelu(hidT + b1), b1 per-partition (m)
                ys_all = mid_pool.tile([64, 512], BF16, tag="ys_all")
                nc.scalar.activation(
                    ys_all[:], hid_all[:], AT.Lrelu, bias=b1c[:], alpha=LEAK
                )

                if phases <= 8:
                    return
                # C7: w~ columns: w_pr = ys_pr^T wr2
                for pr in range(SUPER):
                    nc.tensor.matmul(
                        bat[0:64, pr : pr + 1],
                        lhsT=ys_all[:, pr * 64 : (pr + 1) * 64],
                        rhs=wr2T[:],
                        start=True,
                        stop=True,
                    )
                # C8: w = w~ + br2
                wh = mid_pool.tile([64, 8], BF16, tag="wh")
                nc.vector.tensor_scalar(
                    wh[:], bat[0:64, 0:8], float(BR2_VAL[0]), None, op0=ALU.add
                )
                # C9: g = A^T w
                for pr in range(SUPER):
                    nc.tensor.matmul(
                        bat[0:64, 8 + pr : 9 + pr],
                        lhsT=e_all[:, pr * 64 : (pr + 1) * 64],
                        rhs=wh[:, pr : pr + 1],
                        start=True,
                        stop=True,
                    )
                if phases <= 9:
                    return
                # C10: block-diagonal G: G[p, r, pr] = g_pr[p - (pr%2)*64]
                # when p's global row (r*128+p) falls in pair pr's rows, else 0.
                gG = mid_pool.tile([128, 32], BF16, tag="gG")
                g3 = gG[:].rearrange("p (r pr) -> p r pr", r=4)
                nc.vector.memset(gG[:], 0.0)
                for r in range(4):
                    nc.vector.tensor_copy(
                        g3[0:64, r, 2 * r : 2 * r + 1], bat[0:64, 8 + 2 * r : 9 + 2 * r]
                    )
                    nc.vector.tensor_copy(
                        g3[64:128, r, 2 * r + 1 : 2 * r + 2],
                        bat[0:64, 9 + 2 * r : 10 + 2 * r],
                    )
                # C11: Sw row = ones^T w
                if k1_mm:
                    nc.tensor.matmul(
                        bat[0:1, 88:96], lhsT=ones[:], rhs=wh[:], start=True, stop=True
                    )
                    swS = mid_pool.tile([1, 8], BF16, tag="swS")
                    nc.vector.tensor_copy(swS[:], bat[0:1, 88:96])
                else:
                    swS = None
                # C12: z = Xv^T G, layout [d-in-chunk p, (dc, pr)]; full-K
                # matmuls accumulating over the 4 row-blocks (G is block-diag).
                if z_mm:
                    for dc in range(4):
                        for r in range(4):
                            nc.tensor.matmul(
                                bat[:, 16 + dc * 8 : 24 + dc * 8],
                                lhsT=xv_t[
                                    :, r * 512 + dc * 128 : r * 512 + (dc + 1) * 128
                                ],
                                rhs=g3[:, r, :],
                                start=(r == 0),
                                stop=(r == 3),
                            )
                zS = mid_pool.tile([128, 32], BF16, tag="zS")
                nc.scalar.activation(zS[:], bat[:, 16:48], AT.Copy)
                if phases <= 10:
                    return
                # C13: outT = Wv z + bv Sw, layout [h-in-chunk p, (hc, pr)]
                for hc in range(4):
                    dst = bat[:, 48 + hc * 8 : 56 + hc * 8]
                    for dc in range(4):
                        nc.tensor.matmul(
                            dst,
                            lhsT=wvT[
                                :, dc * 512 + hc * 128 : dc * 512 + (hc + 1) * 128
                            ],
                            rhs=zS[:, dc * 8 : (dc + 1) * 8],
                            start=(dc == 0),
                            stop=(dc == 3) if not k1_mm else False,
                        )
                    if k1_mm:
                        nc.tensor.matmul(
                            dst,
                            lhsT=bvr[0:1, hc * 128 : (hc + 1) * 128],
                            rhs=swS[:],
                            start=False,
                            stop=True,
                        )
                outTs = out_pool.tile([128, 32], F32, tag="outTs")
                nc.scalar.activation(outTs[:], bat[:, 48:80], AT.Copy)
                if phases <= 11:
                    return
                # C14: transpose to natural [pair, h] and store
                outN = out_pool.tile([32, 128], F32, tag="outN")
                for b in range(4):
                    nc.vector.transpose(
                        outN[0:32, b * 32 : (b + 1) * 32],
                        outTs[b * 32 : (b + 1) * 32, 0:32],
                    )
                nc.sync.dma_start(
                    out_d[bass.ts(sb, SUPER), :].rearrange(
                        "pr (hc c) -> hc pr c", hc=4
                    ),
                    outN[:],
                )

            def body(_iv=None):
                load_consts()
                for sb in range(n_sb):
                    emit_superblock(sb)

            if repeat == 1:
                body()
            else:
                with tc.For_i(0, repeat, 1) as _iv:
                    body(_iv)

    nc.compile()
    return nc


def prep_in_maps(query, key, value, Wq, bq, Wk, bk, Wv, bv, Wr1, br1, Wr2, br2):
    """Host-side prep: shard + weight transforms. Returns in_maps list of 8 dicts."""
    s = 1.0 / np.sqrt(np.float32(D))
    wqT = (Wq * s).T.astype(BF16_NP).copy()  # [d, h]
    wkT = Wk.T.astype(BF16_NP).copy()
    wvT = Wv.T.astype(BF16_NP).copy()
    w1vT = (Wr1 @ Wv).T.astype(BF16_NP).copy()  # [d, m]
    wr2T = Wr2.T.astype(BF16_NP).copy()  # [m, 1]
    bqv = (bq * s).astype(np.float32).reshape(4, 128).T.copy()  # [p, hc]
    bkv = bk.astype(np.float32).reshape(4, 128).T.copy()
    b1c = (br1 + Wr1 @ bv).astype(np.float32).reshape(64, 1).copy()
    bvr = bv.astype(BF16_NP).reshape(1, D).copy()
    ones = np.ones((64, 1), dtype=BF16_NP)
    BR2_VAL[0] = float(br2[0])

    in_maps = []
    for c in range(NCORES):
        sl = slice(c * BS_SH, (c + 1) * BS_SH)
        in_maps.append(
            {
                "xq": np.ascontiguousarray(query[sl]).reshape(NPAIR * NSHOT, D),
                "xk": np.ascontiguousarray(key[sl]).reshape(NPAIR * NSHOT, D),
                "xv": np.ascontiguousarray(value[sl]).reshape(NPAIR * NSHOT, D),
                "wqT": wqT,
                "wkT": wkT,
                "wvT": wvT,
                "w1vT": w1vT,
                "wr2T": wr2T,
                "bq": bqv,
                "bk": bkv,
                "b1c": b1c,
                "bvr": bvr,
                "ones": ones,
            }
        )
    return in_maps


_nc_cache = {}


def kernel(**inputs):
    in_maps = prep_in_maps(**{k: np.asarray(v) for k, v in inputs.items()})
    key = ("k", 1, BR2_VAL[0])
    if key not in _nc_cache:
        _nc_cache[key] = build_nc(repeat=1)
    nc = _nc_cache[key]
    res = run_bass_kernel_spmd(nc, in_maps, core_ids=list(range(NCORES)))
    outs = [res.results[c]["out"].reshape(BS_SH, NWAY, D) for c in range(NCORES)]
    return np.concatenate(outs, axis=0).astype(np.float32)
